# revision 6
# baseline (speedup 1.0000x reference)
"""Distributed Trainium2 kernel for nn_Attention_30494267801907.

Multi-head attention (H=16, D=64, N=4096) with RoPE + QK-L2-norm + learned
qk_scale, softmax, and output projection, tensor-parallel over heads on 8
NeuronCores (2 heads per core).

Per-core pipeline (all layouts chosen so no transpose is ever needed on the
hot N^2 path):
  1. prep (natural [seq, feat] layout): rope(x) = x*cos + swap(x)*ss, L2
     norms computed from raw x (rope is a rotation, norm-invariant), rsqrt
     via ACT-sqrt + DVE-reciprocal, qk_scale^2 folded into k. PE-transpose
     128x128 blocks into [feat, seq] bf16 operands.
  2. scores TRANSPOSED: S^T[j,i] = k_hat^T(j) . q_hat^T(i), via matmul with
     K=d=64, both heads packed into disjoint PE row-halves (concurrent).
  3. exp on ACT straight out of PSUM (scale=1/8 folded in), bf16 out. No
     max-subtraction needed: |score| <= qk_scale^2/8 is tiny.
  4. PV with V stationary: out^T[d,i] accumulated over j-blocks in PSUM; a
     ones-column appended to V gives the softmax denominator in row 64 of
     the same matmul.
  5. normalize columns by 1/denominator, giving A^T[f_local, i] bf16.
  6. AllGather A^T over the 8 cores -> full [1024, seq] A^T.
  7. y^T[o_local, i] = wT_local^T @ A^T + bias. Host concatenates the 8
     column-slices and transposes.
"""

import math
from contextlib import ExitStack

import numpy as np

import concourse.bass as bass
import concourse.mybir as mybir
import concourse.tile as tile
from concourse import bacc
from concourse.bass import ds, ts
from concourse.masks import make_identity

F32 = mybir.dt.float32
BF16 = mybir.dt.bfloat16

N_CORES = 8
SEQ = 4096
D = 64          # head dim
HL = 2          # heads per core
FL = HL * D     # local feature cols (128)
DIMF = 1024     # full feature dim
OC = DIMF // N_CORES  # output cols per core (128)
P = 128         # partition tile


def build_graph(seq=SEQ, n_cores=N_CORES):
    nc = bacc.Bacc("TRN2", target_bir_lowering=False, debug=False,
                   num_devices=n_cores)
    NT = seq // P            # seq tiles == j blocks
    IC = min(1024, seq)      # query-chunk per softmax pass
    NIC = seq // IC
    SUBW = min(512, IC)      # matmul moving-operand width
    NSUB = IC // SUBW
    KB = DIMF // P           # K blocks in projection

    q_d = nc.declare_dram_parameter("q", [seq, FL], F32, isOutput=False)
    k_d = nc.declare_dram_parameter("k", [seq, FL], F32, isOutput=False)
    v_d = nc.declare_dram_parameter("v", [seq, FL], F32, isOutput=False)
    cos_d = nc.declare_dram_parameter("cose", [seq, FL], F32, isOutput=False)
    ss_d = nc.declare_dram_parameter("sse", [seq, FL], F32, isOutput=False)
    sv_d = nc.declare_dram_parameter("sv", [1, D], F32, isOutput=False)
    wt_d = nc.declare_dram_parameter("wt", [DIMF, OC], F32, isOutput=False)
    b_d = nc.declare_dram_parameter("bias", [OC, 1], F32, isOutput=False)
    out_d = nc.declare_dram_parameter("out", [OC, seq], F32, isOutput=True)

    with ExitStack() as ctx:
        tc = ctx.enter_context(tile.TileContext(nc))

        const = ctx.enter_context(tc.tile_pool(name="const", bufs=1))
        big = ctx.enter_context(tc.tile_pool(name="big", bufs=1))
        nrm = ctx.enter_context(tc.tile_pool(name="nrm", bufs=1))
        dram = ctx.enter_context(tc.tile_pool(name="dram", bufs=1, space="DRAM"))

        ident = const.tile([P, P], BF16)
        make_identity(nc, ident)
        sv_sb = const.tile([P, D], F32)
        sv_base = sv_d[:, :]
        nc.sync.dma_start(
            out=sv_sb,
            in_=bass.AP(tensor=sv_base.tensor, offset=sv_base.offset,
                        ap=[[0, P], [1, D]]),
        )
        bias_sb = const.tile([OC, 1], F32)
        nc.sync.dma_start(out=bias_sb, in_=b_d[:, :])
        wt_f = const.tile([P, KB, OC], F32)
        nc.sync.dma_start(
            out=wt_f, in_=wt_d[:, :].rearrange("(kb p) o -> p kb o", p=P))
        wt_sb = const.tile([P, KB, OC], BF16)
        nc.vector.tensor_copy(wt_sb, wt_f)

        # persistent operands
        qT = big.tile([P, seq], BF16)          # [f_local, i]
        kT = big.tile([P, seq], BF16)          # [f_local, j]
        v1A = big.tile([P, NT, 2 * D], BF16)   # [j_in_blk, jb, d | ones]
        v1B = big.tile([P, NT, 2 * D], BF16)
        aT = big.tile([P, seq], BF16)          # normalized attn out^T

        nc.vector.memset(v1A, 1.0)
        nc.vector.memset(v1B, 1.0)

        ssq_q = nrm.tile([P, NT, HL], F32)
        ssq_k = nrm.tile([P, NT, HL], F32)
        srt_q = nrm.tile([P, NT, HL], F32)
        srt_k = nrm.tile([P, NT, HL], F32)
        rn_q = nrm.tile([P, NT, HL], F32)
        rn_k = nrm.tile([P, NT, HL], F32)

        cc_in = dram.tile([FL, seq], BF16)
        cc_out = dram.tile([DIMF, seq], BF16, addr_space="Shared")

        def swap_pairs(ap):
            # view with each (2i, 2i+1) free-dim pair swapped
            base = ap
            return bass.AP(tensor=base.tensor, offset=base.offset + 1,
                           ap=[base.ap[0], [2, FL // 2], [-1, 2]])

        # ---------------- prep phase A: load, sumsq, rope ----------------
        with tc.tile_pool(name="prep", bufs=3) as prep, \
             tc.tile_pool(name="roped", bufs=1) as roped, \
             tc.tile_pool(name="tp_psum", bufs=2, space="PSUM") as tpp:
            qR = roped.tile([P, NT, FL], F32)
            kR = roped.tile([P, NT, FL], F32)

            for t in range(NT):
                rows = ds(t * P, P)
                qt = prep.tile([P, FL], F32, tag="qt")
                kt = prep.tile([P, FL], F32, tag="kt")
                vt = prep.tile([P, FL], F32, tag="vt")
                cst = prep.tile([P, FL], F32, tag="cst")
                sst = prep.tile([P, FL], F32, tag="sst")
                nc.sync.dma_start(out=qt, in_=q_d[rows, :])
                nc.sync.dma_start(out=kt, in_=k_d[rows, :])
                nc.sync.dma_start(out=vt, in_=v_d[rows, :])
                nc.sync.dma_start(out=cst, in_=cos_d[rows, :])
                nc.sync.dma_start(out=sst, in_=ss_d[rows, :])

                tmp = prep.tile([P, FL], F32, tag="tmp")
                nc.vector.tensor_mul(tmp, qt, qt)
                nc.vector.tensor_reduce(
                    ssq_q[:, t, :], tmp.rearrange("p (h d) -> p h d", h=HL),
                    axis=mybir.AxisListType.X, op=mybir.AluOpType.add)
                tmp2 = prep.tile([P, FL], F32, tag="tmp2")
                nc.vector.tensor_mul(tmp2, kt, kt)
                nc.vector.tensor_reduce(
                    ssq_k[:, t, :], tmp2.rearrange("p (h d) -> p h d", h=HL),
                    axis=mybir.AxisListType.X, op=mybir.AluOpType.add)

                ra = prep.tile([P, FL], F32, tag="ra")
                nc.vector.tensor_mul(ra, qt, cst)
                nc.vector.tensor_mul(qR[:, t, :], swap_pairs(qt[:, :]), sst)
                nc.vector.tensor_add(qR[:, t, :], qR[:, t, :], ra)
                rb = prep.tile([P, FL], F32, tag="rb")
                nc.vector.tensor_mul(rb, kt, cst)
                nc.vector.tensor_mul(kR[:, t, :], swap_pairs(kt[:, :]), sst)
                nc.vector.tensor_add(kR[:, t, :], kR[:, t, :], rb)

                nc.vector.tensor_copy(v1A[:, t, 0:D], vt[:, 0:D])
                nc.vector.tensor_copy(v1B[:, t, 0:D], vt[:, D:FL])

            # ------------- norms: one batched sqrt per tensor -------------
            nc.scalar.sqrt(srt_q, ssq_q)
            nc.scalar.sqrt(srt_k, ssq_k)
            nc.vector.reciprocal(rn_q, srt_q)
            nc.vector.reciprocal(rn_k, srt_k)

            # ------------- prep phase B: scale + transpose ---------------
            for t in range(NT):
                qs = prep.tile([P, FL], BF16, tag="qs")
                ks = prep.tile([P, FL], BF16, tag="ks")
                for h in range(HL):
                    cols = ds(h * D, D)
                    nc.vector.tensor_scalar_mul(
                        qs[:, cols], qR[:, t, cols], rn_q[:, t, h:h + 1])
                    nc.vector.scalar_tensor_tensor(
                        ks[:, cols], in0=kR[:, t, cols],
                        scalar=rn_k[:, t, h:h + 1], in1=sv_sb,
                        op0=mybir.AluOpType.mult, op1=mybir.AluOpType.mult)
                tq = tpp.tile([P, P], BF16, tag="tq")
                nc.tensor.transpose(tq, qs, ident)
                nc.vector.tensor_copy(qT[:, ts(t, P)], tq)
                tk = tpp.tile([P, P], BF16, tag="tk")
                nc.tensor.transpose(tk, ks, ident)
                nc.vector.tensor_copy(kT[:, ts(t, P)], tk)

        # ---------------- attention ----------------
        with tc.tile_pool(name="s_psum", bufs=1, space="PSUM") as spool, \
             tc.tile_pool(name="o_psum", bufs=2, space="PSUM") as opool, \
             tc.tile_pool(name="epool", bufs=2) as epool, \
             tc.tile_pool(name="rpool", bufs=2) as rpool:
            for ic in range(NIC):
                oA = opool.tile([P, IC], F32, tag="o")
                oB = opool.tile([P, IC], F32, tag="o")
                for jb in range(NT):
                    sAB = spool.tile([P, 2 * IC], F32)
                    for h, cofs in ((0, 0), (1, IC)):
                        hd = ds(h * D, D)
                        for sub in range(NSUB):
                            nc.tensor.matmul(
                                sAB[:, ds(cofs + sub * SUBW, SUBW)],
                                lhsT=kT[hd, ts(jb, P)],
                                rhs=qT[hd, ds(ic * IC + sub * SUBW, SUBW)],
                                start=True, stop=True)
                    eAB = epool.tile([P, 2 * IC], BF16)
                    nc.scalar.activation(
                        eAB, sAB, mybir.ActivationFunctionType.Exp,
                        scale=1.0 / math.sqrt(D))
                    for h, (cofs, op_, v1) in ((0, (0, oA, v1A)),
                                               (1, (IC, oB, v1B))):
                        for sub in range(NSUB):
                            nc.tensor.matmul(
                                op_[:, ds(sub * SUBW, SUBW)],
                                lhsT=v1[:, jb, :],
                                rhs=eAB[:, ds(cofs + sub * SUBW, SUBW)],
                                start=(jb == 0), stop=(jb == NT - 1))
                for h, op_ in ((0, oA), (1, oB)):
                    rec_b = rpool.tile([D, IC], F32)
                    nc.vector.reciprocal(rec_b, op_[D:2 * D, :])
                    nc.vector.tensor_mul(
                        aT[ds(h * D, D), ds(ic * IC, IC)], op_[0:D, :], rec_b)

        # ---------------- all-gather ----------------
        nc.sync.dma_start(out=cc_in[:, :], in_=aT)
        nc.gpsimd.collective_compute(
            "AllGather", mybir.AluOpType.bypass,
            replica_groups=[list(range(n_cores))],
            ins=[cc_in.opt()], outs=[cc_out.opt()])

        # ---------------- projection ----------------
        with tc.tile_pool(name="agp", bufs=KB) as agp, \
             tc.tile_pool(name="y_psum", bufs=2, space="PSUM") as ypp, \
             tc.tile_pool(name="ypool", bufs=3) as ypool:
            ag = []
            for kb in range(KB):
                t_ = agp.tile([P, seq], BF16)
                nc.sync.dma_start(out=t_, in_=cc_out[ds(kb * P, P), :])
                ag.append(t_)
            for nb in range(seq // SUBW):
                py = ypp.tile([OC, SUBW], F32)
                for kb in range(KB):
                    nc.tensor.matmul(
                        py, lhsT=wt_sb[:, kb, :], rhs=ag[kb][:, ts(nb, SUBW)],
                        start=(kb == 0), stop=(kb == KB - 1))
                ysb = ypool.tile([OC, SUBW], F32)
                nc.vector.tensor_scalar_add(ysb, py, bias_sb)
                nc.sync.dma_start(out=out_d[:, ts(nb, SUBW)], in_=ysb)

    nc.compile()
    return nc


def host_inputs(q, k, v, qk_scale, w_out, b_out, n_cores=N_CORES):
    """Shard + derive per-core input maps from the full problem inputs."""
    B, N, dim = q.shape
    assert B == 1 and dim == DIMF

    inv_freq = 1.0 / (10000.0 ** (np.arange(0, D, 2, dtype=np.float64) / D))
    t = np.arange(N, dtype=np.float64)
    freqs = np.outer(t, inv_freq)                       # [N, D/2]
    cos_e = np.repeat(np.cos(freqs), 2, axis=1)         # [N, D]
    ss_e = np.empty((N, D), dtype=np.float64)
    ss_e[:, 0::2] = -np.sin(freqs)
    ss_e[:, 1::2] = np.sin(freqs)
    cose = np.tile(cos_e, (1, HL)).astype(np.float32)   # [N, FL]
    sse = np.tile(ss_e, (1, HL)).astype(np.float32)
    sv = (qk_scale.reshape(-1).astype(np.float64) ** 2).astype(
        np.float32).reshape(1, D)

    in_maps = []
    for c in range(n_cores):
        sl = slice(FL * c, FL * (c + 1))
        in_maps.append({
            "q": np.ascontiguousarray(q[0, :, sl], dtype=np.float32),
            "k": np.ascontiguousarray(k[0, :, sl], dtype=np.float32),
            "v": np.ascontiguousarray(v[0, :, sl], dtype=np.float32),
            "cose": cose,
            "sse": sse,
            "sv": sv,
            "wt": np.ascontiguousarray(w_out[sl, :].T, dtype=np.float32),
            "bias": np.ascontiguousarray(
                b_out[sl].reshape(OC, 1), dtype=np.float32),
        })
    return in_maps


def assemble_output(results, N=SEQ, n_cores=N_CORES):
    out = np.empty((1, N, DIMF), dtype=np.float32)
    for c in range(n_cores):
        out[0, :, FL * c:FL * (c + 1)] = results[c]["out"].T
    return out


_CACHE = {}


def kernel(q, k, v, qk_scale, w_out, b_out):
    from concourse.bass_utils import run_bass_kernel_spmd

    if "nc" not in _CACHE:
        _CACHE["nc"] = build_graph()
    nc = _CACHE["nc"]
    in_maps = host_inputs(q, k, v, qk_scale, w_out, b_out)
    res = run_bass_kernel_spmd(nc, in_maps, core_ids=list(range(N_CORES)))
    return assemble_output(res.results)


# revision 9
# speedup vs baseline: 1.5460x; 1.5460x over previous
"""Distributed Trainium2 kernel for nn_Attention_30494267801907.

Multi-head attention (H=16, D=64, N=4096) with RoPE + QK-L2-norm + learned
qk_scale, softmax, and output projection, tensor-parallel over heads on 8
NeuronCores (2 heads per core).

Per-core pipeline (all layouts chosen so no transpose is ever needed on the
hot N^2 path):
  1. prep (natural [seq, feat] layout): rope(x) = x*cos + swap(x)*ss, L2
     norms computed from raw x (rope is a rotation, norm-invariant), rsqrt
     via ACT-sqrt + DVE-reciprocal, qk_scale^2 folded into k. PE-transpose
     128x128 blocks into [feat, seq] bf16 operands.
  2. scores TRANSPOSED: S^T[j,i] = k_hat^T(j) . q_hat^T(i), via matmul with
     K=d=64, both heads packed into disjoint PE row-halves (concurrent).
  3. exp on ACT straight out of PSUM (scale=1/8 folded in), bf16 out. No
     max-subtraction needed: |score| <= qk_scale^2/8 is tiny.
  4. PV with V stationary: out^T[d,i] accumulated over j-blocks in PSUM; a
     ones-column appended to V gives the softmax denominator in row 64 of
     the same matmul.
  5. normalize columns by 1/denominator, giving A^T[f_local, i] bf16.
  6. AllGather A^T over the 8 cores -> full [1024, seq] A^T.
  7. y^T[o_local, i] = wT_local^T @ A^T + bias. Host concatenates the 8
     column-slices and transposes.
"""

import math
from contextlib import ExitStack

import numpy as np

import concourse.bass as bass
import concourse.mybir as mybir
import concourse.tile as tile
from concourse import bacc
from concourse.bass import ds, ts
from concourse.masks import make_identity

F32 = mybir.dt.float32
BF16 = mybir.dt.bfloat16

N_CORES = 8
SEQ = 4096
D = 64          # head dim
HL = 2          # heads per core
FL = HL * D     # local feature cols (128)
DIMF = 1024     # full feature dim
OC = DIMF // N_CORES  # output cols per core (128)
P = 128         # partition tile


def build_graph(seq=SEQ, n_cores=N_CORES):
    nc = bacc.Bacc("TRN2", target_bir_lowering=False, debug=False,
                   num_devices=n_cores)
    NT = seq // P            # seq tiles == j blocks
    IC = min(512, seq)       # query-chunk per softmax pass
    NIC = seq // IC
    SUBW = min(512, IC)      # matmul moving-operand width
    NSUB = IC // SUBW
    KB = DIMF // P           # K blocks in projection

    q_d = nc.declare_dram_parameter("q", [seq, FL], F32, isOutput=False)
    k_d = nc.declare_dram_parameter("k", [seq, FL], F32, isOutput=False)
    v_d = nc.declare_dram_parameter("v", [seq, FL], F32, isOutput=False)
    cos_d = nc.declare_dram_parameter("cose", [seq, FL], F32, isOutput=False)
    ss_d = nc.declare_dram_parameter("sse", [seq, FL], F32, isOutput=False)
    sv_d = nc.declare_dram_parameter("sv", [1, D], F32, isOutput=False)
    wt_d = nc.declare_dram_parameter("wt", [DIMF, OC], F32, isOutput=False)
    b_d = nc.declare_dram_parameter("bias", [OC, 1], F32, isOutput=False)
    out_d = nc.declare_dram_parameter("out", [OC, seq], F32, isOutput=True)

    with ExitStack() as ctx:
        tc = ctx.enter_context(tile.TileContext(nc))

        const = ctx.enter_context(tc.tile_pool(name="const", bufs=1))
        big = ctx.enter_context(tc.tile_pool(name="big", bufs=1))
        nrm = ctx.enter_context(tc.tile_pool(name="nrm", bufs=1))
        dram = ctx.enter_context(tc.tile_pool(name="dram", bufs=1, space="DRAM"))

        ident = const.tile([P, P], BF16)
        make_identity(nc, ident)
        sv_sb = const.tile([P, D], F32)
        sv_base = sv_d[:, :]
        nc.sync.dma_start(
            out=sv_sb,
            in_=bass.AP(tensor=sv_base.tensor, offset=sv_base.offset,
                        ap=[[0, P], [1, D]]),
        )
        bias_sb = const.tile([OC, 1], F32)
        nc.sync.dma_start(out=bias_sb, in_=b_d[:, :])
        wt_f = const.tile([P, KB, OC], F32)
        nc.sync.dma_start(
            out=wt_f, in_=wt_d[:, :].rearrange("(kb p) o -> p kb o", p=P))
        wt_sb = const.tile([P, KB, OC], BF16)
        nc.vector.tensor_copy(wt_sb, wt_f)

        # persistent operands
        qT = big.tile([P, seq], BF16)          # [f_local, i]
        kT = big.tile([P, seq], BF16)          # [f_local, j]
        v1A = big.tile([P, NT, 2 * D], BF16)   # [j_in_blk, jb, d | ones]
        v1B = big.tile([P, NT, 2 * D], BF16)
        aT = big.tile([P, seq], BF16)          # normalized attn out^T

        nc.vector.memset(v1A, 1.0)
        nc.vector.memset(v1B, 1.0)

        ssq_q = nrm.tile([P, NT, HL], F32)
        ssq_k = nrm.tile([P, NT, HL], F32)
        srt_q = nrm.tile([P, NT, HL], F32)
        srt_k = nrm.tile([P, NT, HL], F32)
        rn_q = nrm.tile([P, NT, HL], F32)
        rn_k = nrm.tile([P, NT, HL], F32)

        def swap_pairs(ap):
            # view with each (2i, 2i+1) free-dim pair swapped
            base = ap
            return bass.AP(tensor=base.tensor, offset=base.offset + 1,
                           ap=[base.ap[0], [2, FL // 2], [-1, 2]])

        # ---------------- prep phase A: load, sumsq, rope ----------------
        with tc.tile_pool(name="prep", bufs=3) as prep, \
             tc.tile_pool(name="roped", bufs=1) as roped, \
             tc.tile_pool(name="tp_psum", bufs=2, space="PSUM") as tpp:
            qR = roped.tile([P, NT, FL], F32)
            kR = roped.tile([P, NT, FL], F32)

            for t in range(NT):
                rows = ds(t * P, P)
                qt = prep.tile([P, FL], F32, tag="qt")
                kt = prep.tile([P, FL], F32, tag="kt")
                vt = prep.tile([P, FL], F32, tag="vt")
                cst = prep.tile([P, FL], F32, tag="cst")
                sst = prep.tile([P, FL], F32, tag="sst")
                nc.sync.dma_start(out=qt, in_=q_d[rows, :])
                nc.sync.dma_start(out=kt, in_=k_d[rows, :])
                nc.sync.dma_start(out=vt, in_=v_d[rows, :])
                nc.sync.dma_start(out=cst, in_=cos_d[rows, :])
                nc.sync.dma_start(out=sst, in_=ss_d[rows, :])

                tmp = prep.tile([P, FL], F32, tag="tmp")
                nc.vector.tensor_mul(tmp, qt, qt)
                nc.vector.tensor_reduce(
                    ssq_q[:, t, :], tmp.rearrange("p (h d) -> p h d", h=HL),
                    axis=mybir.AxisListType.X, op=mybir.AluOpType.add)
                tmp2 = prep.tile([P, FL], F32, tag="tmp2")
                nc.vector.tensor_mul(tmp2, kt, kt)
                nc.vector.tensor_reduce(
                    ssq_k[:, t, :], tmp2.rearrange("p (h d) -> p h d", h=HL),
                    axis=mybir.AxisListType.X, op=mybir.AluOpType.add)

                ra = prep.tile([P, FL], F32, tag="ra")
                nc.vector.tensor_mul(ra, qt, cst)
                nc.vector.tensor_mul(qR[:, t, :], swap_pairs(qt[:, :]), sst)
                nc.vector.tensor_add(qR[:, t, :], qR[:, t, :], ra)
                rb = prep.tile([P, FL], F32, tag="rb")
                nc.vector.tensor_mul(rb, kt, cst)
                nc.vector.tensor_mul(kR[:, t, :], swap_pairs(kt[:, :]), sst)
                nc.vector.tensor_add(kR[:, t, :], kR[:, t, :], rb)

                nc.vector.tensor_copy(v1A[:, t, 0:D], vt[:, 0:D])
                nc.vector.tensor_copy(v1B[:, t, 0:D], vt[:, D:FL])

            # ------------- norms: one batched sqrt per tensor -------------
            nc.scalar.sqrt(srt_q, ssq_q)
            nc.scalar.sqrt(srt_k, ssq_k)
            nc.vector.reciprocal(rn_q, srt_q)
            nc.vector.reciprocal(rn_k, srt_k)

            # ------------- prep phase B: scale + transpose ---------------
            for t in range(NT):
                qs = prep.tile([P, FL], BF16, tag="qs")
                ks = prep.tile([P, FL], BF16, tag="ks")
                for h in range(HL):
                    cols = ds(h * D, D)
                    nc.vector.tensor_scalar_mul(
                        qs[:, cols], qR[:, t, cols], rn_q[:, t, h:h + 1])
                    nc.vector.scalar_tensor_tensor(
                        ks[:, cols], in0=kR[:, t, cols],
                        scalar=rn_k[:, t, h:h + 1], in1=sv_sb,
                        op0=mybir.AluOpType.mult, op1=mybir.AluOpType.mult)
                tq = tpp.tile([P, P], BF16, tag="tq")
                nc.tensor.transpose(tq, qs, ident)
                nc.vector.tensor_copy(qT[:, ts(t, P)], tq)
                tk = tpp.tile([P, P], BF16, tag="tk")
                nc.tensor.transpose(tk, ks, ident)
                nc.vector.tensor_copy(kT[:, ts(t, P)], tk)

        # ---------------- attention + gather + projection ----------------
        n_halves = 2 if NIC >= 2 else 1
        hseq = seq // n_halves
        cc_in = []
        cc_out = []
        for hf in range(n_halves):
            cin_t = dram.tile([FL, hseq], BF16, name=f"cc_in{hf}")
            cout_t = dram.tile([DIMF, hseq], BF16, addr_space="Shared",
                               name=f"cc_out{hf}")
            cc_in.append(cin_t)
            cc_out.append(cout_t)

        with tc.tile_pool(name="s_psum", bufs=2, space="PSUM") as spool, \
             tc.tile_pool(name="o_psum", bufs=2, space="PSUM") as opool, \
             tc.tile_pool(name="y_psum", bufs=2, space="PSUM") as ypp, \
             tc.tile_pool(name="epool", bufs=3) as epool, \
             tc.tile_pool(name="rpool", bufs=2) as rpool, \
             tc.tile_pool(name="agp", bufs=2) as agp, \
             tc.tile_pool(name="ypool", bufs=3) as ypool:

            def emit_gather(half):
                nc.sync.dma_start(out=cc_in[half][:, :],
                                  in_=aT[:, ds(half * hseq, hseq)])
                nc.gpsimd.collective_compute(
                    "AllGather", mybir.AluOpType.bypass,
                    replica_groups=[list(range(n_cores))],
                    ins=[cc_in[half].opt()], outs=[cc_out[half].opt()])

            def emit_proj(half):
                ag = []
                for kb in range(KB):
                    t_ = agp.tile([P, hseq], BF16, tag=f"ag{kb}")
                    nc.sync.dma_start(out=t_,
                                      in_=cc_out[half][ds(kb * P, P), :])
                    ag.append(t_)
                for nb in range(hseq // SUBW):
                    py = ypp.tile([OC, SUBW], F32)
                    for kb in range(KB):
                        nc.tensor.matmul(
                            py, lhsT=wt_sb[:, kb, :],
                            rhs=ag[kb][:, ts(nb, SUBW)],
                            start=(kb == 0), stop=(kb == KB - 1))
                    ysb = ypool.tile([OC, SUBW], F32)
                    nc.vector.tensor_scalar_add(ysb, py, bias_sb)
                    nc.sync.dma_start(
                        out=out_d[:, ds(half * hseq + nb * SUBW, SUBW)],
                        in_=ysb)

            for ic in range(NIC):
                oA = opool.tile([P, IC], F32, tag="o")
                oB = opool.tile([P, IC], F32, tag="o")
                for jb in range(NT):
                    sAB = spool.tile([P, 2 * IC], F32)
                    for h, cofs in ((0, 0), (1, IC)):
                        hd = ds(h * D, D)
                        for sub in range(NSUB):
                            nc.tensor.matmul(
                                sAB[:, ds(cofs + sub * SUBW, SUBW)],
                                lhsT=kT[hd, ts(jb, P)],
                                rhs=qT[hd, ds(ic * IC + sub * SUBW, SUBW)],
                                start=True, stop=True)
                    eAB = epool.tile([P, 2 * IC], BF16)
                    nc.scalar.activation(
                        eAB, sAB, mybir.ActivationFunctionType.Exp,
                        scale=1.0 / math.sqrt(D))
                    for h, (cofs, op_, v1) in ((0, (0, oA, v1A)),
                                               (1, (IC, oB, v1B))):
                        for sub in range(NSUB):
                            nc.tensor.matmul(
                                op_[:, ds(sub * SUBW, SUBW)],
                                lhsT=v1[:, jb, :],
                                rhs=eAB[:, ds(cofs + sub * SUBW, SUBW)],
                                start=(jb == 0), stop=(jb == NT - 1))
                for h, op_ in ((0, oA), (1, oB)):
                    rec_b = rpool.tile([D, IC], F32)
                    nc.vector.reciprocal(rec_b, op_[D:2 * D, :])
                    nc.vector.tensor_mul(
                        aT[ds(h * D, D), ds(ic * IC, IC)], op_[0:D, :], rec_b)
                if n_halves == 2 and ic == NIC // 2 - 1:
                    emit_gather(0)
                    emit_proj(0)
            emit_gather(n_halves - 1)
            emit_proj(n_halves - 1)

    nc.compile()
    return nc


def host_inputs(q, k, v, qk_scale, w_out, b_out, n_cores=N_CORES):
    """Shard + derive per-core input maps from the full problem inputs."""
    B, N, dim = q.shape
    assert B == 1 and dim == DIMF

    inv_freq = 1.0 / (10000.0 ** (np.arange(0, D, 2, dtype=np.float64) / D))
    t = np.arange(N, dtype=np.float64)
    freqs = np.outer(t, inv_freq)                       # [N, D/2]
    cos_e = np.repeat(np.cos(freqs), 2, axis=1)         # [N, D]
    ss_e = np.empty((N, D), dtype=np.float64)
    ss_e[:, 0::2] = -np.sin(freqs)
    ss_e[:, 1::2] = np.sin(freqs)
    cose = np.tile(cos_e, (1, HL)).astype(np.float32)   # [N, FL]
    sse = np.tile(ss_e, (1, HL)).astype(np.float32)
    sv = (qk_scale.reshape(-1).astype(np.float64) ** 2).astype(
        np.float32).reshape(1, D)

    in_maps = []
    for c in range(n_cores):
        sl = slice(FL * c, FL * (c + 1))
        in_maps.append({
            "q": np.ascontiguousarray(q[0, :, sl], dtype=np.float32),
            "k": np.ascontiguousarray(k[0, :, sl], dtype=np.float32),
            "v": np.ascontiguousarray(v[0, :, sl], dtype=np.float32),
            "cose": cose,
            "sse": sse,
            "sv": sv,
            "wt": np.ascontiguousarray(w_out[sl, :].T, dtype=np.float32),
            "bias": np.ascontiguousarray(
                b_out[sl].reshape(OC, 1), dtype=np.float32),
        })
    return in_maps


def assemble_output(results, N=SEQ, n_cores=N_CORES):
    out = np.empty((1, N, DIMF), dtype=np.float32)
    for c in range(n_cores):
        out[0, :, FL * c:FL * (c + 1)] = results[c]["out"].T
    return out


_CACHE = {}


def kernel(q, k, v, qk_scale, w_out, b_out):
    from concourse.bass_utils import run_bass_kernel_spmd

    if "nc" not in _CACHE:
        _CACHE["nc"] = build_graph()
    nc = _CACHE["nc"]
    in_maps = host_inputs(q, k, v, qk_scale, w_out, b_out)
    res = run_bass_kernel_spmd(nc, in_maps, core_ids=list(range(N_CORES)))
    return assemble_output(res.results)
